# revision 41
# baseline (speedup 1.0000x reference)
"""Bass/Trainium2 kernel for nn_Bert_coss (8-core data-parallel over batch).

Computation (per example):
  o1 = relu(X1 @ W.T + b)            [S, H]
  o2 = relu(X2 @ W.T + b)            [S, H]
  o1_doc, o2_doc = mean over S       [H]
  out = sigmoid(relu(concat(o1_doc, o2_doc) @ fd_w.T + fd_b) @ ff_w.T + ff_b)
  scores[s] = o1e[s] . o2_doc   (o1e = o1 ++ o1_doc row), s in 0..S
  att = softmax(scores); output rows 0..S-1 = att[0:S], row S = out.

Key algorithmic simplification: the reference's full [S+1,S+1] co-attention
einsum is only consumed through its last column, so only S+1 dot products
against o2_doc are needed.

Device-side layout: host pre-transposes X to [V, S] so the matmul contraction
dim (V) lands on SBUF partitions with no on-device transpose. o1 is produced
directly in [H, S] layout, which makes the doc-mean a free-axis reduction
(fused into the relu eviction via ACT accum_out) and the score computation a
K=H matvec on the TensorEngine.
"""

import sys

for _p in ("/opt/trn_rl_repo",):
    if _p not in sys.path:
        sys.path.append(_p)

import numpy as np
from contextlib import ExitStack

import concourse.bass as bass
import concourse.tile as tile
from concourse import bacc, mybir
from concourse import bass_utils

B, S, V, H = 64, 512, 768, 256
NCORES = 8
BL = B // NCORES        # examples per core
KV = V // 128           # contraction chunks for the mlp matmul
MH = H // 128           # output-partition chunks of H

F32 = mybir.dt.float32
F32R = mybir.dt.float32r
BF16 = mybir.dt.bfloat16
AF = mybir.ActivationFunctionType


def _build_kernel(tc):
    nc = tc.nc
    x1t = nc.dram_tensor("x1t", [BL, V, S], F32R, kind="ExternalInput").ap()
    x2t = nc.dram_tensor("x2t", [BL, V, S], F32R, kind="ExternalInput").ap()
    wt = nc.dram_tensor("wt", [V, H], F32R, kind="ExternalInput").ap()
    mlp_b = nc.dram_tensor("mlp_b", [H, 1], F32, kind="ExternalInput").ap()
    fdwt = nc.dram_tensor("fdwt", [2 * H, H], F32, kind="ExternalInput").ap()
    fd_b = nc.dram_tensor("fd_b", [H, 1], F32, kind="ExternalInput").ap()
    ffwt = nc.dram_tensor("ffwt", [H, 1], F32, kind="ExternalInput").ap()
    ff_b = nc.dram_tensor("ff_b", [1, 1], F32, kind="ExternalInput").ap()
    out = nc.dram_tensor("out", [BL, S + 1], F32, kind="ExternalOutput").ap()

    with ExitStack() as ctx:
        const = ctx.enter_context(tc.tile_pool(name="const", bufs=1))

        # weight chunks as separate tiles so the k=0 matmul only depends on
        # the first small DMA; chunks beyond k=1 are issued interleaved with
        # the first X-chunk DMAs (DMA completions are FIFO per queue)
        wt_v = wt.rearrange("(k p) h -> p k h", p=128)
        wt_tiles = []
        for k in range(KV):
            wtk = const.tile([128, H], F32R, tag=f"wt{k}")
            wt_tiles.append(wtk)

        def _wt_dma(k):
            nc.sync.dma_start(wt_tiles[k][:], wt_v[:, k, :])

        for k in range(KV):
            _wt_dma(k)
        mlpb_sb = const.tile([128, MH], F32)
        fdwt_sb = const.tile([128, 4 * H], F32)
        fdb_sb = const.tile([128, MH], F32)
        ffwt_sb = const.tile([128, MH], F32)
        ffb_sb = const.tile([1, 1], F32)
        nffb_sb = const.tile([1, 1], F32)

        def _mlpb_dma():
            nc.scalar.dma_start(
                mlpb_sb[:].rearrange("p (m o) -> p m o", m=MH),
                mlp_b.rearrange("(m p) o -> p m o", p=128),
            )

        def _late_const_dmas():
            # parameters only needed by the end-of-kernel head
            nc.scalar.dma_start(
                fdwt_sb[:].rearrange("p (k h) -> p k h", k=4),
                fdwt.rearrange("(k p) h -> p k h", p=128),
            )
            nc.scalar.dma_start(
                fdb_sb[:].rearrange("p (m o) -> p m o", m=MH),
                fd_b.rearrange("(m p) o -> p m o", p=128),
            )
            nc.scalar.dma_start(
                ffwt_sb[:].rearrange("p (m o) -> p m o", m=MH),
                ffwt.rearrange("(m p) o -> p m o", p=128),
            )
            nc.scalar.dma_start(ffb_sb[:], ff_b[:, :])
            nc.vector.tensor_scalar_mul(nffb_sb[:], ffb_sb[:], -1.0)

        # doc-vector raw sums; column b*4 + kc, kc in (o1m0, o1m1, o2m0, o2m1)
        docs_all = const.tile([128, 4 * BL], F32)

        with ExitStack() as mctx:
            xpool = mctx.enter_context(tc.tile_pool(name="x", bufs=5))
            o1pool = mctx.enter_context(tc.tile_pool(name="o1", bufs=4))
            o2pool = mctx.enter_context(tc.tile_pool(name="o2", bufs=2))
            dpool = mctx.enter_context(tc.tile_pool(name="docs", bufs=2))
            apool = mctx.enter_context(tc.tile_pool(name="att", bufs=3))
            mm_ps = mctx.enter_context(tc.tile_pool(name="mmps", bufs=2, space="PSUM"))
            sc_ps = mctx.enter_context(tc.tile_pool(name="scps", bufs=2, space="PSUM"))
            dd_ps = mctx.enter_context(tc.tile_pool(name="ddps", bufs=2, space="PSUM"))

            def finish_example(b, o1T):
                """Scores + softmax + output row for one example, entirely on
                partition 0 / per-example tiles. Token scores use the RAW doc
                sums; the 1/S (and 1/S^2 for the doc-token) scaling is folded
                into the Exp's scale argument."""
                docs = docs_all[:, b * 4 : b * 4 + 4]
                att = apool.tile([1, S], F32)
                ssc = sc_ps.tile([1, S], F32)
                for hk in range(MH):
                    nc.tensor.matmul(
                        ssc[:],
                        docs[:, 2 + hk : 3 + hk],
                        o1T[:, hk * S : (hk + 1) * S],
                        start=(hk == 0),
                        stop=(hk == MH - 1),
                    )
                sdd = dd_ps.tile([1, 1], F32)
                for hk in range(MH):
                    nc.tensor.matmul(
                        sdd[:],
                        docs[:, 2 + hk : 3 + hk],
                        docs[:, hk : hk + 1],
                        start=(hk == 0),
                        stop=(hk == MH - 1),
                    )
                # softmax without max-subtraction (scores are O(25) at most,
                # far inside fp32 exp range)
                s1 = apool.tile([1, 1], F32, name="s1")
                nc.scalar.activation(att[:, 0:S], ssc[:], AF.Exp,
                                     scale=1.0 / S, accum_out=s1[:])
                edd = apool.tile([1, 1], F32, name="edd")
                nc.scalar.activation(edd[:], sdd[:], AF.Exp, scale=1.0 / (S * S))
                stot = apool.tile([1, 1], F32, name="stot")
                nc.vector.tensor_add(stot[:], s1[:], edd[:])
                rs = apool.tile([1, 1], F32, name="rs")
                nc.vector.reciprocal(rs[:], stot[:])
                nc.vector.tensor_scalar_mul(att[:, 0:S], att[:, 0:S], rs[:])
                # SWDGE: keeps the blocking wait off the ACT/SP sequencers
                nc.gpsimd.dma_start(out[b : b + 1, 0:S], att[:, 0:S])

            LAST_NCH = 3          # last example streams in chunk-tiles
            KPC = KV // LAST_NCH
            pending = []
            for b in range(BL):
                o1T = o1pool.tile([128, MH * S], F32)
                for i, xt in enumerate((x1t, x2t)):
                    xt_v = xt[b].rearrange("(k p) s -> p k s", p=128)
                    chunked = b == BL - 1
                    if chunked:
                        xts = []
                        for c in range(LAST_NCH):
                            xc = xpool.tile([128, KPC * S], F32R, name="xc",
                                            tag="xt_sb")
                            nc.sync.dma_start(
                                xc[:].rearrange("p (k s) -> p k s", k=KPC),
                                xt_v[:, c * KPC : (c + 1) * KPC, :],
                            )
                            xts.append(xc)
                    else:
                        xt_sb = xpool.tile([128, KV * S], F32R, tag="xt_sb")
                        nc.sync.dma_start(
                            xt_sb[:].rearrange("p (k s) -> p k s", k=KV), xt_v
                        )
                    if b == 0 and i == 0:
                        _mlpb_dma()
                    if b == 1 and i == 0:
                        _late_const_dmas()
                    pss = [
                        mm_ps.tile([128, S], F32, tag=f"ps{m}", name=f"ps{m}")
                        for m in range(MH)
                    ]
                    for k in range(KV):
                        rhs = (
                            xts[k // KPC][:, (k % KPC) * S : (k % KPC + 1) * S]
                            if chunked
                            else xt_sb[:, k * S : (k + 1) * S]
                        )
                        for m in range(MH):
                            last_mm = nc.tensor.matmul(
                                pss[m][:],
                                wt_tiles[k][:, m * 128 : (m + 1) * 128],
                                rhs,
                                start=(k == 0),
                                stop=(k == KV - 1),
                            )
                    for m in range(MH):
                        kc = i * MH + m
                        if i == 0:
                            dst = o1T[:, m * S : (m + 1) * S]
                        else:
                            o2scr = o2pool.tile([128, S], F32)
                            dst = o2scr[:]
                        nc.scalar.activation(
                            dst,
                            pss[m][:],
                            AF.Relu,
                            bias=mlpb_sb[:, m : m + 1],
                            accum_out=docs_all[:, b * 4 + kc : b * 4 + kc + 1],
                        )

                pending.append((b, o1T))
                if len(pending) > 2:
                    finish_example(*pending.pop(0))
            while pending:
                finish_example(*pending.pop(0))

            # ---- classifier head, batched over the BL examples ----
            # (fd weights are pre-scaled by 1/S on the host, so the raw doc
            # sums are the correct input here)
            docs_v = docs_all[:].rearrange("p (b k) -> p k b", k=4)
            h_sb = dpool.tile([128, MH * BL], F32, name="h_sb")
            for m in range(MH):
                ph = mm_ps.tile([128, BL], F32, tag="ps0", name="ph")
                for kc in range(4):
                    nc.tensor.matmul(
                        ph[:],
                        fdwt_sb[:, kc * H + m * 128 : kc * H + (m + 1) * 128],
                        docs_v[:, kc, :],
                        start=(kc == 0),
                        stop=(kc == 3),
                    )
                nc.scalar.activation(
                    h_sb[:, m * BL : (m + 1) * BL], ph[:], AF.Relu,
                    bias=fdb_sb[:, m : m + 1],
                )
            po = mm_ps.tile([1, BL], F32, tag="ps1", name="po")
            for m in range(MH):
                nc.tensor.matmul(
                    po[:],
                    ffwt_sb[:, m : m + 1],
                    h_sb[:, m * BL : (m + 1) * BL],
                    start=(m == 0),
                    stop=(m == MH - 1),
                )
            # sigmoid(x) = 1/(1+exp(-x)) stays in the Exp table set
            sig_row = dpool.tile([1, BL], F32, name="sig_row")
            nc.scalar.activation(sig_row[:], po[:], AF.Exp,
                                 bias=nffb_sb[0:1, 0:1], scale=-1.0)
            nc.vector.tensor_scalar_add(sig_row[:], sig_row[:], 1.0)
            nc.vector.reciprocal(sig_row[:], sig_row[:])
            # final column of the output: out[:, S] = sigmoid head values
            nc.gpsimd.dma_start(
                out[:, S : S + 1],
                sig_row[0:1, :].rearrange("o (b s) -> o b s", b=BL),
            )


_NC_CACHE = None


def _get_nc():
    global _NC_CACHE
    if _NC_CACHE is None:
        nc = bacc.Bacc("TRN2", target_bir_lowering=False, debug=False,
                       num_devices=NCORES)
        with tile.TileContext(nc) as tc:
            _build_kernel(tc)
        nc.compile()
        _NC_CACHE = nc
    return _NC_CACHE


def kernel(output_1, output_2, mlp_w, mlp_b, fd_w, fd_b, ff_w, ff_b):
    output_1 = np.asarray(output_1, dtype=np.float32)
    output_2 = np.asarray(output_2, dtype=np.float32)
    mlp_w = np.asarray(mlp_w, dtype=np.float32)
    mlp_b = np.asarray(mlp_b, dtype=np.float32)
    fd_w = np.asarray(fd_w, dtype=np.float32)
    fd_b = np.asarray(fd_b, dtype=np.float32)
    ff_w = np.asarray(ff_w, dtype=np.float32)
    ff_b = np.asarray(ff_b, dtype=np.float32)

    # shard over batch, pre-transpose to [V, S]
    x1t = np.ascontiguousarray(
        output_1.reshape(NCORES, BL, S, V).transpose(0, 1, 3, 2)
    )
    x2t = np.ascontiguousarray(
        output_2.reshape(NCORES, BL, S, V).transpose(0, 1, 3, 2)
    )
    wt = np.ascontiguousarray(mlp_w.T)                    # [V, H]
    mlpb = np.ascontiguousarray(mlp_b.reshape(H, 1))
    fdwt = np.ascontiguousarray((fd_w / S).T)             # [2H, H], 1/S folded
                                                          # (doc sums are raw)
    fdb = np.ascontiguousarray(fd_b.reshape(H, 1))
    ffwt = np.ascontiguousarray(ff_w.T)                   # [H, 1]
    ffb = np.ascontiguousarray(ff_b.reshape(1, 1))

    in_maps = [
        dict(x1t=x1t[c], x2t=x2t[c], wt=wt, mlp_b=mlpb, fdwt=fdwt,
             fd_b=fdb, ffwt=ffwt, ff_b=ffb)
        for c in range(NCORES)
    ]
    global _LAST_IN_MAPS
    _LAST_IN_MAPS = in_maps
    nc = _get_nc()
    res = bass_utils.run_bass_kernel_spmd(nc, in_maps, core_ids=list(range(NCORES)))
    att = np.concatenate([res.results[c]["out"] for c in range(NCORES)], axis=0)
    return np.ascontiguousarray(att.T)  # [S+1, B]


# revision 42
# speedup vs baseline: 1.0987x; 1.0987x over previous
"""Bass/Trainium2 kernel for nn_Bert_coss (8-core data-parallel over batch).

Computation (per example):
  o1 = relu(X1 @ W.T + b)            [S, H]
  o2 = relu(X2 @ W.T + b)            [S, H]
  o1_doc, o2_doc = mean over S       [H]
  out = sigmoid(relu(concat(o1_doc, o2_doc) @ fd_w.T + fd_b) @ ff_w.T + ff_b)
  scores[s] = o1e[s] . o2_doc   (o1e = o1 ++ o1_doc row), s in 0..S
  att = softmax(scores); output rows 0..S-1 = att[0:S], row S = out.

Key algorithmic simplification: the reference's full [S+1,S+1] co-attention
einsum is only consumed through its last column, so only S+1 dot products
against o2_doc are needed.

Device-side layout: host pre-transposes X to [V, S] so the matmul contraction
dim (V) lands on SBUF partitions with no on-device transpose. o1 is produced
directly in [H, S] layout, which makes the doc-mean a free-axis reduction
(fused into the relu eviction via ACT accum_out) and the score computation a
K=H matvec on the TensorEngine.
"""

import sys

for _p in ("/opt/trn_rl_repo",):
    if _p not in sys.path:
        sys.path.append(_p)

import numpy as np
from contextlib import ExitStack

import concourse.bass as bass
import concourse.tile as tile
from concourse import bacc, mybir
from concourse import bass_utils

B, S, V, H = 64, 512, 768, 256
NCORES = 8
BL = B // NCORES        # examples per core
KV = V // 128           # contraction chunks for the mlp matmul
MH = H // 128           # output-partition chunks of H

F32 = mybir.dt.float32
F32R = mybir.dt.float32r
BF16 = mybir.dt.bfloat16
AF = mybir.ActivationFunctionType


def _build_kernel(tc):
    nc = tc.nc
    x1t = nc.dram_tensor("x1t", [BL, V, S], F32R, kind="ExternalInput").ap()
    x2t = nc.dram_tensor("x2t", [BL, V, S], F32R, kind="ExternalInput").ap()
    wt = nc.dram_tensor("wt", [V, H], F32R, kind="ExternalInput").ap()
    mlp_b = nc.dram_tensor("mlp_b", [H, 1], F32, kind="ExternalInput").ap()
    fdwt = nc.dram_tensor("fdwt", [2 * H, H], F32, kind="ExternalInput").ap()
    fd_b = nc.dram_tensor("fd_b", [H, 1], F32, kind="ExternalInput").ap()
    ffwt = nc.dram_tensor("ffwt", [H, 1], F32, kind="ExternalInput").ap()
    ff_b = nc.dram_tensor("ff_b", [1, 1], F32, kind="ExternalInput").ap()
    out = nc.dram_tensor("out", [BL, S + 1], F32, kind="ExternalOutput").ap()

    with ExitStack() as ctx:
        const = ctx.enter_context(tc.tile_pool(name="const", bufs=1))

        # weight chunks as separate tiles so the k=0 matmul only depends on
        # the first small DMA; chunks beyond k=1 are issued interleaved with
        # the first X-chunk DMAs (DMA completions are FIFO per queue)
        wt_v = wt.rearrange("(k p) h -> p k h", p=128)
        wt_tiles = []
        for k in range(KV):
            wtk = const.tile([128, H], F32R, tag=f"wt{k}")
            wt_tiles.append(wtk)

        def _wt_dma(k):
            nc.sync.dma_start(wt_tiles[k][:], wt_v[:, k, :])

        for k in range(KV):
            _wt_dma(k)
        mlpb_sb = const.tile([128, MH], F32)
        fdwt_sb = const.tile([128, 4 * H], F32)
        fdb_sb = const.tile([128, MH], F32)
        ffwt_sb = const.tile([128, MH], F32)
        ffb_sb = const.tile([1, 1], F32)

        def _mlpb_dma():
            nc.scalar.dma_start(
                mlpb_sb[:].rearrange("p (m o) -> p m o", m=MH),
                mlp_b.rearrange("(m p) o -> p m o", p=128),
            )

        def _late_const_dmas():
            # parameters only needed by the end-of-kernel head
            nc.scalar.dma_start(
                fdwt_sb[:].rearrange("p (k h) -> p k h", k=4),
                fdwt.rearrange("(k p) h -> p k h", p=128),
            )
            nc.scalar.dma_start(
                fdb_sb[:].rearrange("p (m o) -> p m o", m=MH),
                fd_b.rearrange("(m p) o -> p m o", p=128),
            )
            nc.scalar.dma_start(
                ffwt_sb[:].rearrange("p (m o) -> p m o", m=MH),
                ffwt.rearrange("(m p) o -> p m o", p=128),
            )
            nc.scalar.dma_start(ffb_sb[:], ff_b[:, :])

        # doc-vector raw sums; column b*4 + kc, kc in (o1m0, o1m1, o2m0, o2m1)
        docs_all = const.tile([128, 4 * BL], F32)
        # per-example token scores / doc-token scores, staged on partition 0
        sc_all = const.tile([1, BL * S], F32)
        dd_all = const.tile([1, BL], F32)
        # batched softmax input: row b = example b's S+1 scores
        sc8 = const.tile([BL, S + 1], F32)

        with ExitStack() as mctx:
            xpool = mctx.enter_context(tc.tile_pool(name="x", bufs=5))
            o1pool = mctx.enter_context(tc.tile_pool(name="o1", bufs=2))
            o2pool = mctx.enter_context(tc.tile_pool(name="o2", bufs=2))
            dpool = mctx.enter_context(tc.tile_pool(name="docs", bufs=2))
            mm_ps = mctx.enter_context(tc.tile_pool(name="mmps", bufs=2, space="PSUM"))
            sc_ps = mctx.enter_context(tc.tile_pool(name="scps", bufs=2, space="PSUM"))
            dd_ps = mctx.enter_context(tc.tile_pool(name="ddps", bufs=2, space="PSUM"))

            def do_scores(b, o1T, dsc, after=None):
                ssc = sc_ps.tile([1, S], F32)
                for hk in range(MH):
                    mm = nc.tensor.matmul(
                        ssc[:],
                        dsc[:, 2 + hk : 3 + hk],
                        o1T[:, hk * S : (hk + 1) * S],
                        start=(hk == 0),
                        stop=(hk == MH - 1),
                    )
                    if after is not None:
                        # keep PE from stalling: order these matvecs after the
                        # next example's dense matmuls (order-only edge)
                        tile.add_dep_helper(
                            mm.ins, after.ins, sync=False,
                            reason="pipeline scores behind next example's mlp",
                        )
                sdd = dd_ps.tile([1, 1], F32)
                for hk in range(MH):
                    mm = nc.tensor.matmul(
                        sdd[:],
                        dsc[:, 2 + hk : 3 + hk],
                        dsc[:, hk : hk + 1],
                        start=(hk == 0),
                        stop=(hk == MH - 1),
                    )
                    if after is not None:
                        tile.add_dep_helper(
                            mm.ins, after.ins, sync=False,
                            reason="pipeline scores behind next example's mlp",
                        )
                nc.vector.tensor_copy(sc_all[:, b * S : (b + 1) * S], ssc[:])
                nc.vector.tensor_copy(dd_all[:, b : b + 1], sdd[:])
                # move this example's token scores to softmax row b
                nc.scalar.dma_start(
                    sc8[b : b + 1, 0:S], sc_all[0:1, b * S : (b + 1) * S]
                )

            NCH = 3               # xt arrives as 3 chunk-tiles
            KPC = KV // NCH       # k-chunks per tile
            prev = None
            for b in range(BL):
                o1T = o1pool.tile([128, MH * S], F32)
                for i, xt in enumerate((x1t, x2t)):
                    xt_v = xt[b].rearrange("(k p) s -> p k s", p=128)
                    xt_sb = xpool.tile([128, KV * S], F32R)
                    nc.sync.dma_start(
                        xt_sb[:].rearrange("p (k s) -> p k s", k=KV), xt_v
                    )
                    if b == 0 and i == 0:
                        _mlpb_dma()
                    if b == 1 and i == 0:
                        _late_const_dmas()
                    pss = [
                        mm_ps.tile([128, S], F32, tag=f"ps{m}", name=f"ps{m}")
                        for m in range(MH)
                    ]
                    for k in range(KV):
                        for m in range(MH):
                            last_mm = nc.tensor.matmul(
                                pss[m][:],
                                wt_tiles[k][:, m * 128 : (m + 1) * 128],
                                xt_sb[:, k * S : (k + 1) * S],
                                start=(k == 0),
                                stop=(k == KV - 1),
                            )
                    for m in range(MH):
                        kc = i * MH + m
                        if i == 0:
                            dst = o1T[:, m * S : (m + 1) * S]
                        else:
                            o2scr = o2pool.tile([128, S], F32)
                            dst = o2scr[:]
                        nc.scalar.activation(
                            dst,
                            pss[m][:],
                            AF.Relu,
                            bias=mlpb_sb[:, m : m + 1],
                            accum_out=docs_all[:, b * 4 + kc : b * 4 + kc + 1],
                        )

                if prev is not None:
                    do_scores(*prev, after=last_mm)
                # per-example scaled doc vectors: [o1d0, o1d1, o2d0, o2d1]
                dsc = dpool.tile([128, 4], F32)
                nc.vector.tensor_scalar_mul(
                    dsc[:], docs_all[:, b * 4 : b * 4 + 4], 1.0 / S
                )
                prev = (b, o1T, dsc)
            do_scores(*prev)

        # ---- head (batched over the BL examples) ----
        with ExitStack() as hctx:
            hpool = hctx.enter_context(tc.tile_pool(name="head", bufs=2))
            h_ps = hctx.enter_context(tc.tile_pool(name="hps", bufs=2, space="PSUM"))
            o_ps = hctx.enter_context(tc.tile_pool(name="ops", bufs=1, space="PSUM"))
            spool = hctx.enter_context(tc.tile_pool(name="smax", bufs=1))

            docs_sc = hpool.tile([128, 4 * BL], F32)
            nc.vector.tensor_scalar_mul(docs_sc[:], docs_all[:], 1.0 / S)
            docs_v = docs_sc[:].rearrange("p (b k) -> p k b", k=4)

            h_sb = hpool.tile([128, MH * BL], F32)
            for m in range(MH):
                ph = h_ps.tile([128, BL], F32)
                for kc in range(4):
                    nc.tensor.matmul(
                        ph[:],
                        fdwt_sb[:, kc * H + m * 128 : kc * H + (m + 1) * 128],
                        docs_v[:, kc, :],
                        start=(kc == 0),
                        stop=(kc == 3),
                    )
                nc.scalar.activation(
                    h_sb[:, m * BL : (m + 1) * BL],
                    ph[:],
                    AF.Relu,
                    bias=fdb_sb[:, m : m + 1],
                )
            po = o_ps.tile([1, BL], F32)
            for m in range(MH):
                nc.tensor.matmul(
                    po[:],
                    ffwt_sb[:, m : m + 1],
                    h_sb[:, m * BL : (m + 1) * BL],
                    start=(m == 0),
                    stop=(m == MH - 1),
                )
            sig_row = hpool.tile([1, BL], F32)
            nc.scalar.activation(sig_row[:], po[:], AF.Sigmoid, bias=ffb_sb[0:1, 0:1])

            # ---- softmax over the S+1 scores, batched across examples ----
            # (token scores already gathered into sc8 rows per example)
            nc.scalar.dma_start(
                sc8[:, S : S + 1], dd_all[0:1, :].rearrange("o (b s) -> o b s", b=BL)
            )
            sig8 = spool.tile([BL, 1], F32)
            nc.scalar.dma_start(
                sig8[:], sig_row[0:1, :].rearrange("o (b s) -> o b s", b=BL)
            )

            nmx = spool.tile([BL, 1], F32)
            nc.vector.reduce_max(nmx[:], sc8[:], axis=mybir.AxisListType.X, negate=True)
            ex = spool.tile([BL, S + 1], F32)
            sm = spool.tile([BL, 1], F32)
            nc.scalar.activation(ex[:], sc8[:], AF.Exp, bias=nmx[:], accum_out=sm[:])
            rs = spool.tile([BL, 1], F32)
            nc.vector.reciprocal(rs[:], sm[:])
            att = spool.tile([BL, S + 1], F32)
            nc.vector.tensor_scalar_mul(att[:], ex[:], rs[:])
            nc.vector.tensor_copy(att[:, S : S + 1], sig8[:])
            nc.scalar.dma_start(out[:, :], att[:])


_NC_CACHE = None


def _get_nc():
    global _NC_CACHE
    if _NC_CACHE is None:
        nc = bacc.Bacc("TRN2", target_bir_lowering=False, debug=False,
                       num_devices=NCORES)
        with tile.TileContext(nc) as tc:
            _build_kernel(tc)
        nc.compile()
        _NC_CACHE = nc
    return _NC_CACHE


def kernel(output_1, output_2, mlp_w, mlp_b, fd_w, fd_b, ff_w, ff_b):
    output_1 = np.asarray(output_1, dtype=np.float32)
    output_2 = np.asarray(output_2, dtype=np.float32)
    mlp_w = np.asarray(mlp_w, dtype=np.float32)
    mlp_b = np.asarray(mlp_b, dtype=np.float32)
    fd_w = np.asarray(fd_w, dtype=np.float32)
    fd_b = np.asarray(fd_b, dtype=np.float32)
    ff_w = np.asarray(ff_w, dtype=np.float32)
    ff_b = np.asarray(ff_b, dtype=np.float32)

    # shard over batch, pre-transpose to [V, S]
    x1t = np.ascontiguousarray(
        output_1.reshape(NCORES, BL, S, V).transpose(0, 1, 3, 2)
    )
    x2t = np.ascontiguousarray(
        output_2.reshape(NCORES, BL, S, V).transpose(0, 1, 3, 2)
    )
    wt = np.ascontiguousarray(mlp_w.T)                    # [V, H]
    mlpb = np.ascontiguousarray(mlp_b.reshape(H, 1))
    fdwt = np.ascontiguousarray(fd_w.T)                   # [2H, H]
    fdb = np.ascontiguousarray(fd_b.reshape(H, 1))
    ffwt = np.ascontiguousarray(ff_w.T)                   # [H, 1]
    ffb = np.ascontiguousarray(ff_b.reshape(1, 1))

    in_maps = [
        dict(x1t=x1t[c], x2t=x2t[c], wt=wt, mlp_b=mlpb, fdwt=fdwt,
             fd_b=fdb, ffwt=ffwt, ff_b=ffb)
        for c in range(NCORES)
    ]
    global _LAST_IN_MAPS
    _LAST_IN_MAPS = in_maps
    nc = _get_nc()
    res = bass_utils.run_bass_kernel_spmd(nc, in_maps, core_ids=list(range(NCORES)))
    att = np.concatenate([res.results[c]["out"] for c in range(NCORES)], axis=0)
    return np.ascontiguousarray(att.T)  # [S+1, B]


# revision 43
# speedup vs baseline: 1.3779x; 1.2542x over previous
"""Bass/Trainium2 kernel for nn_Bert_coss (8-core data-parallel over batch).

Computation (per example):
  o1 = relu(X1 @ W.T + b)            [S, H]
  o2 = relu(X2 @ W.T + b)            [S, H]
  o1_doc, o2_doc = mean over S       [H]
  out = sigmoid(relu(concat(o1_doc, o2_doc) @ fd_w.T + fd_b) @ ff_w.T + ff_b)
  scores[s] = o1e[s] . o2_doc   (o1e = o1 ++ o1_doc row), s in 0..S
  att = softmax(scores); output rows 0..S-1 = att[0:S], row S = out.

Key algorithmic simplification: the reference's full [S+1,S+1] co-attention
einsum is only consumed through its last column, so only S+1 dot products
against o2_doc are needed.

Device-side layout: host pre-transposes X to [V, S] so the matmul contraction
dim (V) lands on SBUF partitions with no on-device transpose. o1 is produced
directly in [H, S] layout, which makes the doc-mean a free-axis reduction
(fused into the relu eviction via ACT accum_out) and the score computation a
K=H matvec on the TensorEngine.
"""

import sys

for _p in ("/opt/trn_rl_repo",):
    if _p not in sys.path:
        sys.path.append(_p)

import numpy as np
from contextlib import ExitStack

import concourse.bass as bass
import concourse.tile as tile
from concourse import bacc, mybir
from concourse import bass_utils

B, S, V, H = 64, 512, 768, 256
NCORES = 8
BL = B // NCORES        # examples per core
KV = V // 128           # contraction chunks for the mlp matmul
MH = H // 128           # output-partition chunks of H

F32 = mybir.dt.float32
F32R = mybir.dt.float32r
F16 = mybir.dt.float16
BF16 = mybir.dt.bfloat16
AF = mybir.ActivationFunctionType


def _build_kernel(tc):
    nc = tc.nc
    x1t = nc.dram_tensor("x1t", [BL, V, S], F16, kind="ExternalInput").ap()
    x2t = nc.dram_tensor("x2t", [BL, V, S], F16, kind="ExternalInput").ap()
    wt = nc.dram_tensor("wt", [V, H], F16, kind="ExternalInput").ap()
    mlp_b = nc.dram_tensor("mlp_b", [H, 1], F32, kind="ExternalInput").ap()
    fdwt = nc.dram_tensor("fdwt", [2 * H, H], F32, kind="ExternalInput").ap()
    fd_b = nc.dram_tensor("fd_b", [H, 1], F32, kind="ExternalInput").ap()
    ffwt = nc.dram_tensor("ffwt", [H, 1], F32, kind="ExternalInput").ap()
    ff_b = nc.dram_tensor("ff_b", [1, 1], F32, kind="ExternalInput").ap()
    out = nc.dram_tensor("out", [BL, S + 1], F32, kind="ExternalOutput").ap()

    with ExitStack() as ctx:
        const = ctx.enter_context(tc.tile_pool(name="const", bufs=1))

        # weight chunks as separate tiles so the k=0 matmul only depends on
        # the first small DMA; chunks beyond k=1 are issued interleaved with
        # the first X-chunk DMAs (DMA completions are FIFO per queue)
        wt_v = wt.rearrange("(k p) h -> p k h", p=128)
        wt_tiles = []
        for k in range(KV):
            wtk = const.tile([128, H], F16, tag=f"wt{k}")
            wt_tiles.append(wtk)

        def _wt_dma(k):
            nc.sync.dma_start(wt_tiles[k][:], wt_v[:, k, :])

        for k in range(KV):
            _wt_dma(k)
        mlpb_sb = const.tile([128, MH], F32)
        fdwt_sb = const.tile([128, 4 * H], F32)
        fdb_sb = const.tile([128, MH], F32)
        ffwt_sb = const.tile([128, MH], F32)
        ffb_sb = const.tile([1, 1], F32)

        def _mlpb_dma():
            nc.scalar.dma_start(
                mlpb_sb[:].rearrange("p (m o) -> p m o", m=MH),
                mlp_b.rearrange("(m p) o -> p m o", p=128),
            )

        def _late_const_dmas():
            # parameters only needed by the end-of-kernel head
            nc.scalar.dma_start(
                fdwt_sb[:].rearrange("p (k h) -> p k h", k=4),
                fdwt.rearrange("(k p) h -> p k h", p=128),
            )
            nc.scalar.dma_start(
                fdb_sb[:].rearrange("p (m o) -> p m o", m=MH),
                fd_b.rearrange("(m p) o -> p m o", p=128),
            )
            nc.scalar.dma_start(
                ffwt_sb[:].rearrange("p (m o) -> p m o", m=MH),
                ffwt.rearrange("(m p) o -> p m o", p=128),
            )
            nc.scalar.dma_start(ffb_sb[:], ff_b[:, :])

        # doc-vector raw sums; column b*4 + kc, kc in (o1m0, o1m1, o2m0, o2m1)
        docs_all = const.tile([128, 4 * BL], F32)
        # per-example token scores / doc-token scores, staged on partition 0
        sc_all = const.tile([1, BL * S], F32)
        dd_all = const.tile([1, BL], F32)
        # batched softmax input: row b = example b's S+1 scores
        sc8 = const.tile([BL, S + 1], F32)

        with ExitStack() as mctx:
            xpool = mctx.enter_context(tc.tile_pool(name="x", bufs=5))
            o1pool = mctx.enter_context(tc.tile_pool(name="o1", bufs=2))
            o2pool = mctx.enter_context(tc.tile_pool(name="o2", bufs=2))
            dpool = mctx.enter_context(tc.tile_pool(name="docs", bufs=2))
            mm_ps = mctx.enter_context(tc.tile_pool(name="mmps", bufs=2, space="PSUM"))
            sc_ps = mctx.enter_context(tc.tile_pool(name="scps", bufs=2, space="PSUM"))
            dd_ps = mctx.enter_context(tc.tile_pool(name="ddps", bufs=2, space="PSUM"))

            def do_scores(b, o1T, dsc, after=None):
                ssc = sc_ps.tile([1, S], F32)
                for hk in range(MH):
                    mm = nc.tensor.matmul(
                        ssc[:],
                        dsc[:, 2 + hk : 3 + hk],
                        o1T[:, hk * S : (hk + 1) * S],
                        start=(hk == 0),
                        stop=(hk == MH - 1),
                    )
                    if after is not None:
                        # keep PE from stalling: order these matvecs after the
                        # next example's dense matmuls (order-only edge)
                        tile.add_dep_helper(
                            mm.ins, after.ins, sync=False,
                            reason="pipeline scores behind next example's mlp",
                        )
                sdd = dd_ps.tile([1, 1], F32)
                for hk in range(MH):
                    mm = nc.tensor.matmul(
                        sdd[:],
                        dsc[:, 2 + hk : 3 + hk],
                        dsc[:, hk : hk + 1],
                        start=(hk == 0),
                        stop=(hk == MH - 1),
                    )
                    if after is not None:
                        tile.add_dep_helper(
                            mm.ins, after.ins, sync=False,
                            reason="pipeline scores behind next example's mlp",
                        )
                nc.vector.tensor_copy(sc_all[:, b * S : (b + 1) * S], ssc[:])
                nc.vector.tensor_copy(dd_all[:, b : b + 1], sdd[:])
                # move this example's token scores to softmax row b
                nc.scalar.dma_start(
                    sc8[b : b + 1, 0:S], sc_all[0:1, b * S : (b + 1) * S]
                )

            NCH = 3               # xt arrives as 3 chunk-tiles
            KPC = KV // NCH       # k-chunks per tile
            prev = None
            for b in range(BL):
                o1T = o1pool.tile([128, MH * S], F16)
                for i, xt in enumerate((x1t, x2t)):
                    xt_v = xt[b].rearrange("(k p) s -> p k s", p=128)
                    xt_sb = xpool.tile([128, KV * S], F16)
                    nc.sync.dma_start(
                        xt_sb[:].rearrange("p (k s) -> p k s", k=KV), xt_v
                    )
                    if b == 0 and i == 0:
                        _mlpb_dma()
                    if b == 1 and i == 0:
                        _late_const_dmas()
                    pss = [
                        mm_ps.tile([128, S], F32, tag=f"ps{m}", name=f"ps{m}")
                        for m in range(MH)
                    ]
                    for k in range(KV):
                        for m in range(MH):
                            last_mm = nc.tensor.matmul(
                                pss[m][:],
                                wt_tiles[k][:, m * 128 : (m + 1) * 128],
                                xt_sb[:, k * S : (k + 1) * S],
                                start=(k == 0),
                                stop=(k == KV - 1),
                            )
                    for m in range(MH):
                        kc = i * MH + m
                        if i == 0:
                            dst = o1T[:, m * S : (m + 1) * S]
                        else:
                            o2scr = o2pool.tile([128, S], F32)
                            dst = o2scr[:]
                        nc.scalar.activation(
                            dst,
                            pss[m][:],
                            AF.Relu,
                            bias=mlpb_sb[:, m : m + 1],
                            accum_out=docs_all[:, b * 4 + kc : b * 4 + kc + 1],
                        )

                if prev is not None:
                    do_scores(*prev, after=last_mm)
                # per-example scaled doc vectors: [o1d0, o1d1, o2d0, o2d1]
                dsc = dpool.tile([128, 4], F16)
                nc.vector.tensor_scalar_mul(
                    dsc[:], docs_all[:, b * 4 : b * 4 + 4], 1.0 / S
                )
                prev = (b, o1T, dsc)
            do_scores(*prev)

        # ---- head (batched over the BL examples) ----
        with ExitStack() as hctx:
            hpool = hctx.enter_context(tc.tile_pool(name="head", bufs=2))
            h_ps = hctx.enter_context(tc.tile_pool(name="hps", bufs=2, space="PSUM"))
            o_ps = hctx.enter_context(tc.tile_pool(name="ops", bufs=1, space="PSUM"))
            spool = hctx.enter_context(tc.tile_pool(name="smax", bufs=1))

            docs_sc = hpool.tile([128, 4 * BL], F32)
            nc.vector.tensor_scalar_mul(docs_sc[:], docs_all[:], 1.0 / S)
            docs_v = docs_sc[:].rearrange("p (b k) -> p k b", k=4)

            h_sb = hpool.tile([128, MH * BL], F32)
            for m in range(MH):
                ph = h_ps.tile([128, BL], F32)
                for kc in range(4):
                    nc.tensor.matmul(
                        ph[:],
                        fdwt_sb[:, kc * H + m * 128 : kc * H + (m + 1) * 128],
                        docs_v[:, kc, :],
                        start=(kc == 0),
                        stop=(kc == 3),
                    )
                nc.scalar.activation(
                    h_sb[:, m * BL : (m + 1) * BL],
                    ph[:],
                    AF.Relu,
                    bias=fdb_sb[:, m : m + 1],
                )
            po = o_ps.tile([1, BL], F32)
            for m in range(MH):
                nc.tensor.matmul(
                    po[:],
                    ffwt_sb[:, m : m + 1],
                    h_sb[:, m * BL : (m + 1) * BL],
                    start=(m == 0),
                    stop=(m == MH - 1),
                )
            sig_row = hpool.tile([1, BL], F32)
            nc.scalar.activation(sig_row[:], po[:], AF.Sigmoid, bias=ffb_sb[0:1, 0:1])

            # ---- softmax over the S+1 scores, batched across examples ----
            # (token scores already gathered into sc8 rows per example)
            nc.scalar.dma_start(
                sc8[:, S : S + 1], dd_all[0:1, :].rearrange("o (b s) -> o b s", b=BL)
            )
            sig8 = spool.tile([BL, 1], F32)
            nc.scalar.dma_start(
                sig8[:], sig_row[0:1, :].rearrange("o (b s) -> o b s", b=BL)
            )

            nmx = spool.tile([BL, 1], F32)
            nc.vector.reduce_max(nmx[:], sc8[:], axis=mybir.AxisListType.X, negate=True)
            ex = spool.tile([BL, S + 1], F32)
            sm = spool.tile([BL, 1], F32)
            nc.scalar.activation(ex[:], sc8[:], AF.Exp, bias=nmx[:], accum_out=sm[:])
            rs = spool.tile([BL, 1], F32)
            nc.vector.reciprocal(rs[:], sm[:])
            att = spool.tile([BL, S + 1], F32)
            nc.vector.tensor_scalar_mul(att[:], ex[:], rs[:])
            nc.vector.tensor_copy(att[:, S : S + 1], sig8[:])
            nc.scalar.dma_start(out[:, :], att[:])


_NC_CACHE = None


def _get_nc():
    global _NC_CACHE
    if _NC_CACHE is None:
        nc = bacc.Bacc("TRN2", target_bir_lowering=False, debug=False,
                       num_devices=NCORES)
        with tile.TileContext(nc) as tc:
            _build_kernel(tc)
        nc.compile()
        _NC_CACHE = nc
    return _NC_CACHE


def kernel(output_1, output_2, mlp_w, mlp_b, fd_w, fd_b, ff_w, ff_b):
    output_1 = np.asarray(output_1, dtype=np.float32)
    output_2 = np.asarray(output_2, dtype=np.float32)
    mlp_w = np.asarray(mlp_w, dtype=np.float32)
    mlp_b = np.asarray(mlp_b, dtype=np.float32)
    fd_w = np.asarray(fd_w, dtype=np.float32)
    fd_b = np.asarray(fd_b, dtype=np.float32)
    ff_w = np.asarray(ff_w, dtype=np.float32)
    ff_b = np.asarray(ff_b, dtype=np.float32)

    # shard over batch, pre-transpose to [V, S]
    x1t = np.ascontiguousarray(
        output_1.reshape(NCORES, BL, S, V).transpose(0, 1, 3, 2)
    ).astype(np.float16)
    x2t = np.ascontiguousarray(
        output_2.reshape(NCORES, BL, S, V).transpose(0, 1, 3, 2)
    ).astype(np.float16)
    wt = np.ascontiguousarray(mlp_w.T).astype(np.float16)  # [V, H]
    mlpb = np.ascontiguousarray(mlp_b.reshape(H, 1))
    fdwt = np.ascontiguousarray(fd_w.T)                   # [2H, H]
    fdb = np.ascontiguousarray(fd_b.reshape(H, 1))
    ffwt = np.ascontiguousarray(ff_w.T)                   # [H, 1]
    ffb = np.ascontiguousarray(ff_b.reshape(1, 1))

    in_maps = [
        dict(x1t=x1t[c], x2t=x2t[c], wt=wt, mlp_b=mlpb, fdwt=fdwt,
             fd_b=fdb, ffwt=ffwt, ff_b=ffb)
        for c in range(NCORES)
    ]
    global _LAST_IN_MAPS
    _LAST_IN_MAPS = in_maps
    nc = _get_nc()
    res = bass_utils.run_bass_kernel_spmd(nc, in_maps, core_ids=list(range(NCORES)))
    att = np.concatenate([res.results[c]["out"] for c in range(NCORES)], axis=0)
    return np.ascontiguousarray(att.T)  # [S+1, B]
